# revision 27
# baseline (speedup 1.0000x reference)
"""TRN2 Bass kernel for nn_MultiHeadSelfAttention_15822659518596.

Key algebraic fact: in the reference, softmax and V are dead code — the
output is

    out[b,i,:] = (scores[b,i].reshape(S*H)) @ W_fc.T + b_fc
    scores[b,i,j,n] = (q[b,i,n,:] . k[b,j,n,:]) / 8

which collapses into dense GEMMs without materializing the (B,S,S,H)
score tensor:

    Kf_b = x_b @ Wk.T                      (S, D)   [c = n*64+kk head-major]
    M_b[c,o]  = sum_j Kf_b[j,c] * Wfc[o, j*8+n(c)] / 8      (D, OH)
    G_b[d,o]  = sum_c Wq[c,d]   * M_b[c,o]                  (D, OH)
    outT[o,i] = sum_d G_b[d,o]  * xT_b[d,i]                 (OH, S)

Sharding: 8 cores = (4 batches) x (2 halves of the fc output dim o).
Each core computes outT[o_half, S] for its (b, h) — no collectives
(SBUF collectives are broken in this bass build, and HBM-HBM
collectives add traffic on the shared HBM rather than saving it).

Measured machine model (from NTFF traces of v3-v6):
  - Framework preamble: no DMA can issue before ~7.1us, PE free at ~7.5.
  - Two HWDGE queues only (scalar, sync).  Sync starts ~8us and
    sustains ~210KB/us; scalar starts ~10us at ~190KB/us.  Transfers
    <512KB and issue chains beyond the 4-semaphore window lose
    throughput, so all bulk transfers here are >=512KB.
  - The HAM clock manager halves the PE clock after ~2-3.5us of PE
    idle and needs several us of sustained activity to restore it.
    Schedules with a front-loaded DMA phase and a sized "warm-up
    matmul" bridge are fragile (the warm-up's own duration is bimodal
    with the clock), so the PE must be kept busy with real dribbled
    work: x streams in j-halves and stage 1 doubles as the clock
    bridge; stage-2 head-blocks are interleaved in measured arrival
    order so no idle gap exceeds ~2us.
  - Critical path = last wfc byte (~36.5us, balanced across queues) +
    tail [16 matmuls of the last head + G + stage 4 + last out chunk].

v7 layout summary:
  sync  : wkq(d0,d1) | x(j-half1) A | x(j-half1) B | h1 h3 h5 h6b h7
  scalar: wkq(d2,d3) | x(j-half2) A | x(j-half2) B | h0 h2 h4 h6a
  PE    : warmup x80 | Kf jt0-15 | s2 h1 h0 h3 [G0] h2 h5 [G1] h4 h6
          [G2] h7 [G3] | G casts | stage 4 (+ per-half output DMAs)
All four M accumulators stay live in PSUM (2 banks) so head-blocks can
be appended in any arrival order; M/G chains run start=False into
memset banks (per-element has_written semantics make any interleave of
the disjoint col-group chains correct).
"""

import ml_dtypes
import numpy as np

import concourse.bass as bass
import concourse.tile as tile
from concourse import mybir, bacc
from concourse.bass_utils import run_bass_kernel_spmd

B, S, D, H = 4, 2048, 512, 8
DK = D // H            # 64
OH = D // 2            # 256, per-core o-half
NC = 8                 # cores
F32 = mybir.dt.float32
BF16 = mybir.dt.bfloat16
COPY = mybir.ActivationFunctionType.Identity

_CACHE = {}


def _build_program(with_bias: bool):
    """One SPMD Bass program; per-core tensors differ only in data."""
    nc = bacc.Bacc("TRN2", target_bir_lowering=False, debug=False, num_devices=NC)

    # xp packs x_b.T's (128, 1024) blocks as
    # [d0j0 | d1j0 | d0j1 | d1j1 | d2j0 | d3j0 | d2j1 | d3j1]
    # (j0 = cols 0:1024, j1 = cols 1024:2048) so each queue moves x in
    # two 512KB contiguous transfers and stage 1 can start on j-half 1.
    xp = nc.dram_tensor("xp", [128, 8 * 1024], BF16, kind="ExternalInput")
    # wkp packs Wk.T's four (128, 512) d-tiles side by side; wqp packs
    # Wq's four (128, 512) c-tiles likewise.  Separate tensors so the
    # stage-1-critical prefix is only wk + x j-half1 (1.5MB); wq rides
    # later in the stream (needed first at G_0, ~24us).
    wkp = nc.dram_tensor("wkp", [128, 4 * 512], BF16, kind="ExternalInput")
    wqp = nc.dram_tensor("wqp", [128, 4 * 512], BF16, kind="ExternalInput")
    wfc = nc.dram_tensor("wfc", [H, 128, 16 * OH], BF16, kind="ExternalInput")
    if with_bias:
        colsum = nc.dram_tensor("colsum", [1, H * OH], BF16, kind="ExternalInput")
        bkrow = nc.dram_tensor("bkrow", [1, D], BF16, kind="ExternalInput")
        bq_col = nc.dram_tensor("bq_col", [128, 4], BF16, kind="ExternalInput")
        bfc_row = nc.dram_tensor("bfc_row", [1, OH], F32, kind="ExternalInput")
    outT = nc.dram_tensor("outT", [OH, S], BF16, kind="ExternalOutput")

    with tile.TileContext(nc) as tc:
        with tc.tile_pool(name="xt", bufs=4) as p_xt, \
             tc.tile_pool(name="wk", bufs=2) as p_wk, \
             tc.tile_pool(name="kf", bufs=16) as p_kf, \
             tc.tile_pool(name="wf", bufs=8) as p_wf, \
             tc.tile_pool(name="m", bufs=4) as p_m, \
             tc.tile_pool(name="g", bufs=4) as p_g, \
             tc.tile_pool(name="ob", bufs=2) as p_ob, \
             tc.tile_pool(name="bias", bufs=1) as p_bias, \
             tc.tile_pool(name="psA", bufs=3, space="PSUM") as psA, \
             tc.tile_pool(name="psM", bufs=(3 if with_bias else 2), space="PSUM") as psM, \
             tc.tile_pool(name="psG", bufs=2, space="PSUM") as psG:

            engs = [nc.scalar, nc.sync]

            # ---- PE warm-up bridge: 48 free dummy matmuls (~107ns
            # each at 1.2GHz, ~55ns once the HAM grants full clock at
            # 10.2-12.8us) end ~11.3-12.9us.  Data-gated bridge dummies
            # are appended below the DMA plan: their stationary operands
            # are the arriving wk/x tiles, so slow-DMA cores pace the PE
            # with the stream (max idle ~1.2us — under the ~2us de-ramp
            # threshold) instead of gambling on a fixed warm-up length
            # (v10: per-core DMA jitter de-ramped one core for +4us).
            t_wu = p_bias.tile([128, 128], BF16, tag="wu")
            nc.vector.memset(t_wu[:], 0.0)
            pw = psA.tile([128, D], F32, tag="acc")
            for _ in range(48):
                nc.tensor.matmul(pw[:, :128], t_wu[:], t_wu[:],
                                 start=True, stop=True)

            # ---- DMA plan (v8): the DMA fabric serves roughly ONE
            # queue's burst at a time (v7 trace: sync monopolized
            # ~430KB/us for 10us while scalar moved nothing, then they
            # swapped), so a two-queue "balanced" plan scrambles arrival
            # order.  A single queue alone sustains ~380-430KB/us — the
            # fabric total, not the queue, is the constraint.  So: the
            # ENTIRE input stream rides the sync queue in exact
            # consumption order [wkq, x j-half1, x j-half2, h0..h7]; the
            # PE then self-paces behind the stream with no reordering
            # risk.  Scalar carries only bias + output.
            t_wk = p_wk.tile([128, 2048], BF16, tag="wk")
            t_wq = p_wk.tile([128, 2048], BF16, tag="wk")
            # x j-halves live in four separate tiles so stage-1's early
            # tiles depend only on their own DMA (no tile-level dep on
            # the later j-half transfers).
            t_xA1 = p_xt.tile([128, 2048], BF16, tag="xt")
            t_xA2 = p_xt.tile([128, 2048], BF16, tag="xt")
            t_xB1 = p_xt.tile([128, 2048], BF16, tag="xt")
            t_xB2 = p_xt.tile([128, 2048], BF16, tag="xt")
            wfs = []
            for n in range(H):
                t_w = p_wf.tile([128, 16 * OH], BF16, tag="wf", name=f"wf{n}")
                wfs.append(t_w)
            nc.sync.dma_start(t_wk[:], wkp[:])
            nc.sync.dma_start(t_xA1[:], xp[:, 0:2048])
            nc.sync.dma_start(t_xB1[:], xp[:, 4096:6144])
            nc.sync.dma_start(t_xA2[:], xp[:, 2048:4096])
            nc.sync.dma_start(t_xB2[:], xp[:, 6144:8192])
            nc.sync.dma_start(t_wq[:], wqp[:])
            for n in range(H):
                nc.sync.dma_start(wfs[n][:], wfc[n][:, :])
            # data-gated PE bridge (see warm-up comment): 4 dummies per
            # arriving stage-1-critical tile
            for t_gate in (t_wk, t_xA1, t_xB1):
                for _ in range(4):
                    nc.tensor.matmul(pw[:, :128], t_gate[:, :128], t_wu[:],
                                     start=True, stop=True)
            if with_bias:
                t_bk = p_bias.tile([1, D], BF16, tag="bk")
                nc.scalar.dma_start(t_bk[:], bkrow[:])
                t_cs = p_bias.tile([1, H * OH], BF16, tag="cs")
                nc.scalar.dma_start(t_cs[:], colsum[:])
                t_bq = p_bias.tile([128, 4], BF16, tag="bq")
                nc.scalar.dma_start(t_bq[:], bq_col[:])
                t_bfc = p_bias.tile([1, OH], F32, tag="bfc")
                nc.scalar.dma_start(t_bfc[:], bfc_row[:])
                t_ones = p_bias.tile([1, 512], BF16, tag="ones")
                nc.vector.memset(t_ones[:], 1.0)

            wks = [t_wk[:, i * 512:(i + 1) * 512] for i in range(4)]
            wqs = [t_wq[:, i * 512:(i + 1) * 512] for i in range(4)]

            xtiles = [[t_xA1, t_xA2], [t_xB1, t_xB2]]

            def xsl(di, a, w):
                """AP slice of x_b.T[di*128:(di+1)*128, a:a+w] in the
                packed tiles; [a, a+w) must stay within one j-half."""
                t = xtiles[di // 2][a // 1024]
                off = (di % 2) * 1024 + (a % 1024)
                return t[:, off:off + w]

            # ---- stage 1: Kf[j, c] (16 j-tiles), Kf = x @ Wk.T.
            # jt 0-7 need only x j-half 1 (region-level deps let them
            # run while j-half 2 streams).  Kf casts alternate vector /
            # scalar: one engine's ~690ns/cast would gate the 432ns/tile
            # matmul rate through the 2-deep psA rotation.
            kfs = []
            for jt in range(16):
                pk = psA.tile([128, D], F32, tag="acc")
                for di in range(4):
                    nc.tensor.matmul(
                        pk[:], xsl(di, jt * 128, 128), wks[di],
                        start=(di == 0), stop=(di == 3))
                t_kf = p_kf.tile([128, D], BF16, tag="kf", name=f"kf{jt}")
                if jt % 2 == 0:
                    nc.vector.tensor_copy(t_kf[:], pk[:])
                else:
                    nc.scalar.activation(t_kf[:], pk[:], COPY)
                kfs.append(t_kf)

            # ---- stage 2+3: all four M accumulators live in PSUM so
            # head-blocks append in arrival order (sync heads early, the
            # paired scalar head after); G_u folds in as soon as pair u
            # is complete, overlapped with the next head's matmuls.
            pmA = psM.tile([128, 512], F32, tag="pm")
            pmB = psM.tile([128, 512], F32, tag="pm")
            nc.vector.memset(pmA[:], 0.0)
            nc.vector.memset(pmB[:], 0.0)
            pg0 = psG.tile([128, 512], F32, tag="pg")
            pg1 = psG.tile([128, 512], F32, tag="pg")
            pgs = [pg0[:, :OH], pg0[:, OH:], pg1[:, :OH], pg1[:, OH:]]
            nc.vector.memset(pg0[:], 0.0)
            nc.vector.memset(pg1[:], 0.0)
            if with_bias:
                pv = psM.tile([1, OH], F32, tag="pv")

            def pm_ap(u, half):
                base = pmA if u < 2 else pmB
                c0 = (u % 2) * OH
                return base[half * 64:half * 64 + 64, c0:c0 + OH]

            def s2_head(n, last_of_pair):
                u = n // 2
                half = n % 2
                tp = (0, 0) if half == 0 else (0, 64)
                for jt in range(16):
                    nc.tensor.matmul(
                        pm_ap(u, half)[:], kfs[jt][:, n * 64:(n + 1) * 64],
                        wfs[n][:, jt * OH:(jt + 1) * OH],
                        start=False,
                        stop=(last_of_pair and not with_bias and jt == 15),
                        tile_position=tp, skip_group_check=True)
                if last_of_pair and with_bias:
                    n0, n1 = 2 * u, 2 * u + 1
                    nc.tensor.matmul(
                        pm_ap(u, 0)[:], t_bk[0:1, n0 * 64:(n0 + 1) * 64],
                        t_cs[0:1, n0 * OH:(n0 + 1) * OH],
                        start=False, stop=False, tile_position=(0, 0),
                        skip_group_check=True)
                    nc.tensor.matmul(
                        pm_ap(u, 1)[:], t_bk[0:1, n1 * 64:(n1 + 1) * 64],
                        t_cs[0:1, n1 * OH:(n1 + 1) * OH],
                        start=False, stop=True, tile_position=(0, 64),
                        skip_group_check=True)

            t_ms = [None] * 4

            def m_cast(u):
                t_m = p_m.tile([128, OH], BF16, tag="m", name=f"m{u}")
                base = pmA if u < 2 else pmB
                c0 = (u % 2) * OH
                nc.vector.tensor_copy(t_m[:], base[:, c0:c0 + OH])
                if with_bias:
                    nc.tensor.matmul(pv[:], t_bq[:, u:u + 1], t_m[:],
                                     start=(u == 0), stop=(u == 3))
                t_ms[u] = t_m

            def g_fold(u):
                for dc in range(4):
                    nc.tensor.matmul(
                        pgs[dc][:], wqs[u][:, dc * 128:(dc + 1) * 128],
                        t_ms[u][:], start=False, stop=(u == 3),
                        skip_group_check=True)

            # heads arrive in order h0..h7 (~2.6us apart); G_u sits one
            # head-block after pair u completes so the M cast overlaps
            # the intervening head's matmuls.
            s2_head(0, False); s2_head(1, True); m_cast(0)
            s2_head(2, False); g_fold(0)
            s2_head(3, True); m_cast(1)
            s2_head(4, False); g_fold(1)
            s2_head(5, True); m_cast(2)
            s2_head(6, False); g_fold(2)
            s2_head(7, True); m_cast(3)
            g_fold(3)

            # ---- G psum -> bf16 SBUF per d-quadrant, alternating
            # vector/scalar so stage 4's first chain isn't cast-gated.
            gs = []
            for dc in range(4):
                t_g = p_g.tile([128, OH], BF16, tag="g", name=f"g{dc}")
                if dc % 2 == 0:
                    nc.vector.tensor_copy(t_g[:], pgs[dc][:])
                else:
                    nc.scalar.activation(t_g[:], pgs[dc][:], COPY)
                gs.append(t_g)
            if with_bias:
                t_vb = p_bias.tile([1, OH], BF16, tag="vb")
                v_f = p_bias.tile([1, OH], F32, tag="vf")
                nc.vector.tensor_add(v_f[:], pv[:], t_bfc[:])
                nc.vector.tensor_copy(t_vb[:], v_f[:])

            # ---- stage 4: outT[o, i] = sum_d G[d,o] * xT[d,i] (+ v[o]).
            # Output leaves per (oc, ic-pair) as 512KB DMAs; the final
            # chunk is split across both queues to halve the end tail.
            for oc in range(2):
                t_o = p_ob.tile([128, S], BF16, tag="ob", name=f"ob{oc}")
                for ic in range(4):
                    po = psA.tile([128, 512], F32, tag="acc")
                    if with_bias:
                        nc.tensor.matmul(
                            po[:], t_vb[0:1, oc * 128:(oc + 1) * 128],
                            t_ones[0:1, :], start=True, stop=False)
                    for dc in range(4):
                        nc.tensor.matmul(
                            po[:], gs[dc][:, oc * 128:(oc + 1) * 128],
                            xsl(dc, ic * 512, 512),
                            start=(not with_bias and dc == 0), stop=(dc == 3))
                    if ic % 2 == 0:
                        nc.vector.tensor_copy(
                            t_o[:, ic * 512:(ic + 1) * 512], po[:])
                    else:
                        nc.scalar.activation(
                            t_o[:, ic * 512:(ic + 1) * 512], po[:], COPY)
                    if ic % 2 == 1:
                        # each 512KB output half-chunk leaves as two
                        # 256KB DMAs, one per queue (sync is warm and
                        # idle by now; scalar has a cold-start lag)
                        c0 = (ic - 1) * 512
                        nc.sync.dma_start(
                            outT[oc * 128:(oc + 1) * 128, c0:c0 + 512],
                            t_o[:, c0:c0 + 512])
                        nc.scalar.dma_start(
                            outT[oc * 128:(oc + 1) * 128, c0 + 512:c0 + 1024],
                            t_o[:, c0 + 512:c0 + 1024])
    nc.compile()
    return nc


def _prep_inputs(x, W_qkv, b_qkv, W_fc, b_fc):
    """Host-side sharding/layout prep. O(bytes) only — no GEMM work."""
    x = np.ascontiguousarray(x, dtype=np.float32)
    W_qkv = np.asarray(W_qkv, dtype=np.float32)
    b_qkv = np.asarray(b_qkv, dtype=np.float32)
    W_fc = np.asarray(W_fc, dtype=np.float32)
    b_fc = np.asarray(b_fc, dtype=np.float32)
    with_bias = bool(np.any(b_qkv) or np.any(b_fc))

    wqkv = W_qkv.reshape(H, 3, DK, D)  # [n, {q,k,v}, kk, d]
    wq_cd = wqkv[:, 0].reshape(D, D)                      # [c, d]
    wkT = wqkv[:, 1].reshape(D, D).T                      # [d, c]
    # pack the four (128, 512) tiles of each side-by-side -> (128, 2048)
    wkp = np.ascontiguousarray(
        wkT.reshape(4, 128, 512).transpose(1, 0, 2).reshape(128, 2048)
    ).astype(ml_dtypes.bfloat16)
    wqp = np.ascontiguousarray(
        wq_cd.reshape(4, 128, 512).transpose(1, 0, 2).reshape(128, 2048)
    ).astype(ml_dtypes.bfloat16)
    bq = b_qkv.reshape(H, 3, DK)
    bq_c = np.ascontiguousarray(bq[:, 0].reshape(D))      # c-order
    bk_c = np.ascontiguousarray(bq[:, 1].reshape(D))
    bq_col = np.ascontiguousarray(bq_c.reshape(4, 128).T).astype(ml_dtypes.bfloat16)
    bkrow = bk_c.reshape(1, D).astype(ml_dtypes.bfloat16)

    Wfc_s = W_fc * (1.0 / 8.0)
    # per o-half h: [n, jj, t, o] layout, plus per-head column sums
    wfc_h, cs_h, bfc_h = [], [], []
    for h in range(2):
        A = Wfc_s[h * OH:(h + 1) * OH, :]                  # (256, 16384)
        arr = np.ascontiguousarray(A.T).reshape(S, H, OH).transpose(1, 0, 2)  # [n,j,o]
        cs = np.ascontiguousarray(arr.sum(axis=1)).reshape(1, H * OH)
        arr2 = np.ascontiguousarray(
            arr.reshape(H, 16, 128, OH).transpose(0, 2, 1, 3)  # [n, jj, t, o]
        ).reshape(H, 128, 16 * OH).astype(ml_dtypes.bfloat16)
        wfc_h.append(arr2)
        cs_h.append(cs.astype(ml_dtypes.bfloat16))
        bfc_h.append(np.ascontiguousarray(b_fc[h * OH:(h + 1) * OH].reshape(1, OH)))

    # x_b.T packed: [d0j0|d1j0|d0j1|d1j1|d2j0|d3j0|d2j1|d3j1], each
    # block (128, 1024)
    xp_b = []
    for b in range(B):
        xT = x[b].T  # (512, 2048)
        blocks = []
        for dpair in (0, 1):
            for jh in (0, 1):
                for di in (2 * dpair, 2 * dpair + 1):
                    blocks.append(xT[di * 128:(di + 1) * 128,
                                     jh * 1024:(jh + 1) * 1024])
        xp_b.append(np.ascontiguousarray(
            np.concatenate(blocks, axis=1)).astype(ml_dtypes.bfloat16))

    in_maps = []
    for c in range(NC):
        b, h = c // 2, c % 2
        m = {
            "xp": xp_b[b],
            "wkp": wkp,
            "wqp": wqp,
            "wfc": wfc_h[h],
        }
        if with_bias:
            m.update({
                "colsum": cs_h[h],
                "bkrow": bkrow,
                "bq_col": bq_col,
                "bfc_row": bfc_h[h],
            })
        in_maps.append(m)
    return in_maps, with_bias


def _run(in_maps, with_bias, trace=False, **kw):
    key = ("nc", with_bias)
    if key not in _CACHE:
        _CACHE[key] = _build_program(with_bias)
    return run_bass_kernel_spmd(
        _CACHE[key], in_maps, core_ids=list(range(NC)), trace=trace, **kw)


def _assemble(results):
    out = np.empty((B, S, D), dtype=np.float32)
    for c in range(NC):
        b, h = c // 2, c % 2
        out[b, :, h * OH:(h + 1) * OH] = results[c]["outT"].T.astype(np.float32)
    return out


def kernel(x, W_qkv, b_qkv, W_fc, b_fc):
    in_maps, with_bias = _prep_inputs(x, W_qkv, b_qkv, W_fc, b_fc)
    res = _run(in_maps, with_bias, trace=False)
    return _assemble(res.results)


def kernel_traced(x, W_qkv, b_qkv, W_fc, b_fc):
    """Like kernel() but returns (out, BassKernelResults) with NTFF trace."""
    import os
    os.environ.setdefault("BASS_PERFETTO_PROFILE_ALL_CORES", "1")
    _install_ntff_hook_shim()
    in_maps, with_bias = _prep_inputs(x, W_qkv, b_qkv, W_fc, b_fc)
    res = _run(in_maps, with_bias, trace=True)
    return _assemble(res.results), res


def _install_ntff_hook_shim():
    """The agent image's antenv lacks axon_hooks; provide it so
    run_bass_kernel_spmd(trace=True) can reach the NTFF profiler."""
    import sys, types
    if "antenv.axon_hooks" in sys.modules:
        return
    try:
        from trn_agent_boot.trn_boot import _ntff_profile_via_ctypes
    except ImportError:
        return
    mod = types.ModuleType("antenv.axon_hooks")
    _hook = [None]
    mod.set_axon_ntff_profile_hook = lambda h: _hook.__setitem__(0, h)
    mod.get_axon_ntff_profile_hook = lambda: _hook[0]
    import antenv
    sys.modules["antenv.axon_hooks"] = mod
    antenv.axon_hooks = mod
    so = "/opt/axon/libaxon_pjrt.so"
    try:
        hook = _ntff_profile_via_ctypes(so)
    except OSError:
        hook = None
    mod.set_axon_ntff_profile_hook(hook)
